# revision 36
# baseline (speedup 1.0000x reference)
"""Trainium2 Bass kernel for causal multi-head attention with RoPE.

Full-input contract: kernel(**inputs) takes the unsharded tensors and
returns the full [B, S, D] output. Internally the work is sharded over
8 NeuronCores: cores 0-3 compute batch 0, cores 4-7 batch 1; within a
batch group each core owns 4 of the 16 heads (tensor-parallel over
heads). Each core computes its partial output-projection contribution
[S, D]; the host sums the 4 partials per batch and adds the biases
that commute with attention (wo_b, and wv_b which passes through the
softmax untouched because attention weights sum to 1).

All matmul operands are bf16 (same 1 cycle/row PE rate as fp32r but
half the SBUF/DMA footprint), with fp32 PSUM accumulation.

Every intermediate stays in SBUF. The three stages are interleaved per
512-query chunk n: project Q/K/V for chunk n (+RoPE), run causal
attention for query chunk n against keys 0..n, then emit the output
projection for chunk n-1.

Schedule notes (from perfetto analysis):
- the PE runs at 0.83ns/col (vs 0.42 full speed) until ~3us of
  continuous busy, so the kernel opens with 13 dummy matmuls over a
  memset scratch tile that burn the initial DMA-wait window and hand
  the real chunk-0 chain a fully-ramped PE.
- startup: strict need-order on the two HWDGE issue streams. sync(SP):
  wq pieces (2k,2k,4k,4k,4k slices of the contraction) then xn
  prefetches; scalar(ACT): x0 pieces, the first half of wk, and the
  rope tables, with the wk tail/wv/wo issued at program positions
  behind chunk-0's rope activations (the ACT queue blocks there until
  the Q psums complete), because queue descriptors are FIFO per queue
  and the full-speed Q chain consumes the (wq, x0) head at ~300GB/s.
- out projection rows are staged into one [128, D] SBUF slab per
  128-row block and written with a single DMA (16 output DMAs total,
  the last one split in two so the final transfer overlaps the last
  copies), issued from the SP sequencer.
- the softmax normalize chain (ones-matmul partition collapse +
  reciprocal + multiply) of pass p is deferred into pass p+1 (across
  the chunk boundary for the last pass) so the PE never waits on the
  DVE's denominator adds at a pass boundary.
- for chunks >= 1 the first attention pass's first three score tiles
  are emitted right after the Q projections+RoPE, so their exps run
  on ACT during the K/V projections and the pass starts with a hot
  pipeline.
"""

import os
import sys

sys.path.insert(0, "/opt/trn_rl_repo")

import numpy as np
import ml_dtypes

B = 2
S = 2048
D = 2048
H = 16
DK = 128
N_CORES = 8
HPC = 4          # heads per core
E = HPC * DK     # 512: per-core slice of the model dim
CH = 512         # sequence chunk (query chunk = projection chunk)
NCH = S // CH    # 4 chunks
KO = D // 128    # contraction chunks for the projections
NJ = S // 128    # key tiles
ISQRT_DK = 1.0 / np.sqrt(DK)

_CACHE = {}

last_exec_time_ns = None
last_results = None

# k-slice groups for the startup streams: fine at the head so the first
# matmul starts early, coarser behind (2KB descriptors halve DMA
# efficiency, 4KB are fine)
_HEAD_GROUPS = [(0, 2), (2, 4), (4, 8), (8, 12), (12, 16)]


def _build_program():
    import concourse.mybir as mybir
    import concourse.tile as tile
    from concourse import bacc

    dt = mybir.dt
    F32 = dt.float32
    BF16 = dt.bfloat16
    AF = mybir.ActivationFunctionType

    nc = bacc.Bacc(None, target_bir_lowering=False, debug=True)

    # inputs are host-packed into the exact SBUF layouts so every DMA
    # descriptor is 128 contiguous per-partition segments
    xP = nc.dram_tensor("xP", [NCH, 128, KO, CH], BF16, kind="ExternalInput")
    wqkv = nc.dram_tensor(
        "wqkv", [3, 128, KO, E], BF16, kind="ExternalInput"
    )
    woT = nc.dram_tensor("woT", [128, HPC, D], BF16, kind="ExternalInput")
    bqk = nc.dram_tensor("bqk", [DK, 2 * HPC], F32, kind="ExternalInput")
    # aux: [cc2 | sss | mask | ones] along the free dim
    aux = nc.dram_tensor("aux", [128, 2 * S + 256], BF16, kind="ExternalInput")
    out = nc.dram_tensor("out", [S, D], BF16, kind="ExternalOutput")

    with tile.TileContext(nc) as tc:
        with (
            tc.tile_pool(name="const", bufs=1) as cpool,
            tc.tile_pool(name="w", bufs=1) as wpool,
            tc.tile_pool(name="x", bufs=2) as xpool,
            tc.tile_pool(name="kres", bufs=1) as krpool,
            tc.tile_pool(name="vres", bufs=1) as vpool,
            tc.tile_pool(name="q", bufs=2) as qpool,
            tc.tile_pool(name="rope", bufs=2) as rpool,
            tc.tile_pool(name="p", bufs=1) as ppool,
            tc.tile_pool(name="pl", bufs=2) as plpool,
            tc.tile_pool(name="li", bufs=2) as lipool,
            tc.tile_pool(name="ob", bufs=1) as obpool,
            tc.tile_pool(name="psum", bufs=1, space="PSUM") as pspool,
        ):
            # ---- PE p-state warm-up: the PE runs at 0.83ns/col until
            # ~3us of continuous busy, so burn the DMA-wait window on
            # dummy matmuls over a memset scratch tile; the real chunk-0
            # chain then starts at full speed ----
            scr = cpool.tile([128, CH], BF16, name="scr")
            nc.vector.memset(scr[:], 0)
            warm_ps = pspool.tile([128, CH], F32, tag="po", bufs=2,
                                  name="warm_ps")
            for _ in range(13):
                nc.tensor.matmul(
                    warm_ps[:], scr[:, 0:128], scr[:],
                    start=True, stop=True,
                )

            # ---- tiny constants on the gpsimd SWDGE ----
            bqk_sb = cpool.tile([DK, 2 * HPC], F32, name="bqk_sb")
            nc.gpsimd.dma_start(bqk_sb[:], bqk[:])
            wqkv_sb = wpool.tile([128, 3 * KO, E], BF16, name="wqkv_sb")
            wo_sb = wpool.tile([128, HPC, D], BF16, name="wo_sb")
            aux_sb = cpool.tile([128, 2 * S + 256], BF16, name="aux_sb")
            cc2_sb = aux_sb[:, 0:S]
            sss_sb = aux_sb[:, S : 2 * S]
            mask_sb = aux_sb[:, 2 * S : 2 * S + 128]
            ones_sb = aux_sb[:, 2 * S + 128 : 2 * S + 256]

            def wsl(t, k0, k1=None):
                # k-range slice of weight t (0=q,1=k,2=v) in wqkv_sb
                if k1 is None:
                    k1 = k0 + 1
                return wqkv_sb[:, t * KO + k0 : t * KO + k1, :]

            xn_next = xpool.tile([128, KO, CH], BF16, tag="xn", name="xn0")
            # head of the stream: (wq, x0) k-pairs, interleaved across the
            # two HWDGE sequencers in consumption order
            for g0, g1 in _HEAD_GROUPS:
                nc.sync.dma_start(wsl(0, g0, g1), wqkv[0][:, g0:g1, :])
                nc.scalar.dma_start(
                    xn_next[:, g0:g1, :], xP[0][:, g0:g1, :]
                )
            # Behind the head on scalar: the first half of wk and the rope
            # tables fit in the queue-bandwidth budget before the K chain
            # needs them; everything else (wk tail, wv, wo) is issued at
            # program positions BEHIND chunk-0's rope activations — the
            # ACT queue blocks there until the Q chain's psums complete
            # (~26us in), which keeps the early DMA bandwidth for the
            # (wq, x0) head that the full-speed Q chain consumes at
            # ~300GB/s.
            nc.scalar.dma_start(wsl(1, 0, 4), wqkv[1][:, 0:4, :])
            nc.scalar.dma_start(wsl(1, 4, 8), wqkv[1][:, 4:8, :])
            nc.scalar.dma_start(aux_sb[:], aux[:])
            startup_dmas = [
                (wsl(1, 8, 12), wqkv[1][:, 8:12, :]),
                (wsl(1, 12, 16), wqkv[1][:, 12:16, :]),
            ]
            for g in range(KO // 4):
                gs = (4 * g, 4 * g + 4)
                startup_dmas.append(
                    (wsl(2, *gs), wqkv[2][:, gs[0] : gs[1], :])
                )
            startup_dmas.append((wo_sb[:], woT[:]))

            def issue_startup(k):
                while startup_dmas and k > 0:
                    dst, src = startup_dmas.pop(0)
                    nc.scalar.dma_start(dst, src)
                    k -= 1

            def load_xn(n, engine=None):
                xn = xpool.tile([128, KO, CH], BF16, tag="xn", name=f"xn{n}")
                (engine or nc.sync).dma_start(xn[:], xP[n])
                return xn

            kr = krpool.tile([128, HPC, S], BF16, name="kr")
            vres = vpool.tile([128, NJ, E], BF16, name="vres")

            # persistent slabs with manual ring indices (each SBUF tile
            # costs ~150ns in the end-of-kernel event drain, so churny
            # ring pools are folded into single tiles; the framework's
            # range-granular dependency tracking provides the same
            # pipelining)
            p_slab = ppool.tile([128, 4, CH], BF16, name="p_slab")
            p_ctr = [0]
            ob_slab = obpool.tile([128, 2, D], BF16, name="ob_slab")
            ob_ctr = [0]

            def emit_c(ao_t, n_src, ii, split=False, ring2=False):
                # output projection for rows [n_src*CH + ii*128, +128),
                # staged into one [128, D] slab -> single out DMA (two
                # half DMAs for the very last emission so the final
                # transfer overlaps the last copies). ring2: alternate
                # the psum ring with the score ring — only safe after
                # the last pass, when no scores are in flight — so the
                # chains never wait on the previous slab's ob copies.
                r0 = n_src * CH + ii * 128
                ob = ob_slab[:, ob_ctr[0] % 2, :]
                ob_ctr[0] += 1
                for fc in range(4):
                    tag = "ps" if (ring2 and fc % 2) else "aps"
                    pc = pspool.tile([128, 512], F32, tag=tag, bufs=3)
                    for ec in range(HPC):
                        nc.tensor.matmul(
                            pc[:],
                            ao_t[:, ec, ii * 128 : (ii + 1) * 128],
                            wo_sb[:, ec, fc * 512 : (fc + 1) * 512],
                            start=(ec == 0),
                            stop=(ec == HPC - 1),
                        )
                    if split:
                        # final emission: halve each copy across both
                        # engines and write per-fc, so the very last
                        # transfer is gated by a 256-col copy instead of
                        # a full 512-col one
                        c0 = fc * 512
                        nc.vector.tensor_copy(
                            ob[:, c0 : c0 + 256], pc[:, 0:256]
                        )
                        nc.scalar.activation(
                            ob[:, c0 + 256 : c0 + 512], pc[:, 256:512],
                            AF.Copy,
                        )
                        nc.sync.dma_start(
                            out[r0 : r0 + 128, c0 : c0 + 512],
                            ob[:, c0 : c0 + 512],
                        )
                    elif fc % 2 == 0:
                        # copies alternate DVE/ACT: one engine alone
                        # would serialize the psum-ring recycle
                        nc.vector.tensor_copy(
                            ob[:, fc * 512 : (fc + 1) * 512], pc[:]
                        )
                    else:
                        nc.scalar.activation(
                            ob[:, fc * 512 : (fc + 1) * 512], pc[:], AF.Copy
                        )
                if not split:
                    nc.sync.dma_start(out[r0 : r0 + 128, :], ob[:])

            def do_norm(st, pe=True):
                # collapse the 128 partial-denominator rows with one
                # 128x128 ones matmul, then normalize into the pass's
                # ao. (A GPSIMD partition_all_reduce was tried instead —
                # correct but ~2us/op on HW, +34us total: the GPSIMD is
                # idle but far too slow for the normalize chain.)
                pstot = pspool.tile([128, CH], F32, tag="ps", bufs=3)
                nc.tensor.matmul(
                    pstot[:], ones_sb, st["pl"][:],
                    start=True, stop=True,
                )
                li = lipool.tile([128, CH], F32, tag="li")
                nc.vector.reciprocal_approx_fast(li[:], pstot[:])
                nc.vector.tensor_mul(
                    st["ao"][:, st["m"], :], st["po"][:], li[:]
                )

            ao_prev = None
            deferred = None
            for n in range(NCH):
                nsl = slice(n * CH, (n + 1) * CH)
                xn = xn_next
                if 0 < n and n + 1 < NCH:
                    xn_next = load_xn(n + 1)

                # qc (RoPE'd queries) and ao (normalized attention out)
                # share one per-chunk tile
                qa = qpool.tile(
                    [128, 2 * HPC, CH], BF16, tag="qa", name=f"qa{n}"
                )
                qc = qa[:, 0:HPC, :]
                ao_cur = qa[:, HPC : 2 * HPC, :]

                def rope(pq, bsb, dst):
                    rp = rpool.tile([128, 3, CH], BF16, tag="rp")
                    st0 = rp[:, 0, :]
                    sw = rp[:, 1, :]
                    rot = rp[:, 2, :]
                    nc.scalar.activation(
                        st0, pq[:], AF.Identity, bias=bsb
                    )
                    # RoPE: d-rows packed [even; odd] per head, so the
                    # rotate pair is partition r <-> r+64
                    nc.vector.tensor_copy(sw[0:64, :], st0[64:128, :])
                    nc.vector.tensor_copy(sw[64:128, :], st0[0:64, :])
                    nc.vector.tensor_mul(rot, st0, cc2_sb[:, nsl])
                    nc.vector.tensor_mul(sw, sw, sss_sb[:, nsl])
                    nc.vector.tensor_add(dst, rot, sw)

                # ---- attention machinery for chunk n ----
                njc = 4 * n + 4  # key tiles per pass

                def score_exp(m, jc):
                    t = jc - 4 * n  # >=0 on the diagonal band
                    cs = 128 * t if t >= 0 else 0
                    ps = pspool.tile([128, CH], F32, tag="ps", bufs=3)
                    nc.tensor.matmul(
                        ps[:, cs:CH],
                        kr[:, m, jc * 128 : (jc + 1) * 128],
                        qc[:, m, cs:CH],
                        start=True,
                        stop=True,
                    )
                    p = p_slab[:, p_ctr[0] % 4, :]
                    p_ctr[0] += 1
                    nc.scalar.activation(
                        p[:, cs:CH], ps[:, cs:CH], AF.Exp,
                        scale=float(ISQRT_DK),
                    )
                    if t >= 0:
                        nc.vector.tensor_mul(
                            p[:, cs : cs + 128],
                            p[:, cs : cs + 128],
                            mask_sb,
                        )
                    return (p, jc, cs)

                def psum4():
                    # four simultaneously-live psum tiles borrowed from the
                    # aps(3) + ps(3) rings for chunk-0 k-outer chains
                    return [
                        pspool.tile(
                            [128, CH],
                            F32,
                            tag=("aps" if i < 2 else "ps"),
                            bufs=3,
                            name=f"pk0_{i}",
                        )
                        for i in range(HPC)
                    ]

                warm = []
                # ---- stage A: project chunk n (+RoPE on Q/K) ----
                if n == 0:
                    # k-outer on the first chunk: the PE consumes each
                    # (wq, x) k-pair as its DMAs land instead of stalling
                    # on the full tensors
                    for wt, boff, is_q in ((0, 0, True), (1, HPC, False)):
                        pqs = psum4()
                        for k in range(KO):
                            for m in range(HPC):
                                nc.tensor.matmul(
                                    pqs[m][:],
                                    wqkv_sb[
                                        :, wt * KO + k,
                                        m * DK : (m + 1) * DK,
                                    ],
                                    xn[:, k, :],
                                    start=(k == 0),
                                    stop=(k == KO - 1),
                                )
                        for m in range(HPC):
                            dst = qc[:, m, :] if is_q else kr[:, m, nsl]
                            rope(pqs[m], bqk_sb[:, boff + m : boff + m + 1],
                                 dst)
                            # drain one deferred startup DMA issue behind
                            # each rope activation (see comment above)
                            issue_startup(1)
                        if not is_q:
                            xn_next = load_xn(1, nc.scalar)
                    pvs = psum4()
                    for k in range(KO):
                        for jj in range(CH // 128):
                            nc.tensor.matmul(
                                pvs[jj][:],
                                xn[:, k, jj * 128 : (jj + 1) * 128],
                                wqkv_sb[:, 2 * KO + k, :],
                                start=(k == 0),
                                stop=(k == KO - 1),
                            )
                    for jj in range(CH // 128):
                        nc.scalar.activation(
                            vres[:, n * 4 + jj, :], pvs[jj][:], AF.Copy
                        )
                else:
                    def qk_chain(wt, m):
                        pq = pspool.tile([128, CH], F32, tag="aps", bufs=3)
                        for k in range(KO):
                            nc.tensor.matmul(
                                pq[:],
                                wqkv_sb[
                                    :, wt * KO + k, m * DK : (m + 1) * DK
                                ],
                                xn[:, k, :],
                                start=(k == 0),
                                stop=(k == KO - 1),
                            )
                        return pq

                    for m in range(HPC):
                        pq = qk_chain(0, m)
                        rope(pq, bqk_sb[:, m : m + 1], qc[:, m, :])
                    # pre-warm pass 0's first score tiles: their exps run
                    # on ACT during the K/V projections (~27us of slack),
                    # so pass 0 starts with a hot softmax pipeline
                    for jc in range(3):
                        warm.append(score_exp(0, jc))
                    # previous chunk's last-pass normalize lands here, far
                    # from both its denominator adds and its emit readers
                    if deferred is not None:
                        do_norm(deferred)
                        deferred = None
                    for m in range(HPC):
                        pq = qk_chain(1, m)
                        rope(pq, bqk_sb[:, HPC + m : HPC + m + 1],
                             kr[:, m, nsl])
                    for jj in range(CH // 128):
                        pvp = pspool.tile([128, E], F32, tag="aps", bufs=3)
                        for k in range(KO):
                            nc.tensor.matmul(
                                pvp[:],
                                xn[:, k, jj * 128 : (jj + 1) * 128],
                                wqkv_sb[:, 2 * KO + k, :],
                                start=(k == 0),
                                stop=(k == KO - 1),
                            )
                        nc.scalar.activation(
                            vres[:, n * 4 + jj, :], pvp[:], AF.Copy
                        )

                # ---- stage B: attention for query chunk n ----
                # ---- stage C (interleaved): out-proj for chunk n-1 ----
                # software pipeline: scores run up to three tiles ahead of
                # the P@V matmuls (and are pre-warmed across pass
                # boundaries); the normalize chain of pass m is deferred
                # into pass m+1 so the PE never waits on the DVE's
                # denominator adds
                for m in range(HPC):
                    po = pspool.tile([128, CH], F32, tag="po", bufs=2)
                    pl = plpool.tile([128, CH], BF16, tag="pl")
                    cur = {"po": po, "pl": pl, "m": m, "ao": ao_cur}

                    def emit_pv(p, jc, cs):
                        # denominator accumulate (DVE, bf16) + P@V (PE)
                        if jc == 0:
                            nc.vector.tensor_copy(pl[:, cs:CH], p[:, cs:CH])
                        else:
                            nc.vector.tensor_add(
                                pl[:, cs:CH], pl[:, cs:CH], p[:, cs:CH]
                            )
                        nc.tensor.matmul(
                            po[:, cs:CH],
                            vres[:, jc, m * DK : (m + 1) * DK],
                            p[:, cs:CH],
                            start=(jc == 0),
                            stop=(jc == njc - 1),
                        )

                    if deferred is not None:
                        do_norm(deferred)
                        deferred = None

                    pending = warm
                    warm = []
                    for jc in range(len(pending), njc):
                        pending.append(score_exp(m, jc))
                        if len(pending) > 2:
                            emit_pv(*pending.pop(0))
                    for it in pending:
                        emit_pv(*it)

                    # pre-warm the next pass's first score tiles: they keep
                    # the PE busy while the last denominator adds drain on
                    # the DVE, and cover the exp latency of the next pass
                    if m + 1 < HPC:
                        for jc in range(min(3, njc)):
                            warm.append(score_exp(m + 1, jc))
                        deferred = cur
                    elif n + 1 < NCH:
                        # defer even the last pass's normalize into the
                        # next chunk's projection phase
                        deferred = cur
                    else:
                        do_norm(cur, pe=True)

                    if ao_prev is not None:
                        emit_c(ao_prev, n - 1, m)

                ao_prev = ao_cur

            for ii in range(4):
                emit_c(ao_prev, NCH - 1, ii, split=(ii == 3), ring2=True)

    nc.compile()
    return nc


def _rope_tables():
    inv_freq = 1.0 / (10000.0 ** (np.arange(0, DK, 2, dtype=np.float64) / DK))
    pos = np.arange(S, dtype=np.float64)
    freqs = pos[:, None] * inv_freq[None, :]  # [S, DK/2]
    cos_t = np.cos(freqs).T.astype(np.float32)  # [64, S]
    sin_t = np.sin(freqs).T.astype(np.float32)
    cc2 = np.ascontiguousarray(np.concatenate([cos_t, cos_t], axis=0))
    sss = np.ascontiguousarray(np.concatenate([-sin_t, sin_t], axis=0))
    return cc2, sss


def _bf16(a):
    return np.ascontiguousarray(a.astype(ml_dtypes.bfloat16))


def kernel(
    x, wq_w, wq_b, wk_w, wk_b, wv_w, wv_b, wo_w, wo_b
) -> np.ndarray:
    global last_exec_time_ns, last_results
    from concourse.bass_utils import run_bass_kernel_spmd

    if "nc" not in _CACHE:
        _CACHE["nc"] = _build_program()
    nc = _CACHE["nc"]

    x = np.asarray(x, dtype=np.float32)
    wq_w = np.asarray(wq_w, dtype=np.float32)
    wk_w = np.asarray(wk_w, dtype=np.float32)
    wv_w = np.asarray(wv_w, dtype=np.float32)
    wo_w = np.asarray(wo_w, dtype=np.float32)
    wq_b = np.asarray(wq_b, dtype=np.float32)
    wk_b = np.asarray(wk_b, dtype=np.float32)
    wv_b = np.asarray(wv_b, dtype=np.float32)
    wo_b = np.asarray(wo_b, dtype=np.float32)

    cc2, sss = _rope_tables()
    r_idx = np.arange(128)[:, None]
    c_idx = np.arange(128)[None, :]
    mask_np = (r_idx <= c_idx).astype(np.float32)
    ones = np.ones((128, 128), dtype=np.float32)
    aux_np = _bf16(
        np.concatenate(
            [cc2, sss, mask_np, ones], axis=1
        )
    )
    # within each head, pack d-rows as [even dims; odd dims]
    perm = np.concatenate([np.arange(0, DK, 2), np.arange(1, DK, 2)])

    # pack to the on-chip layouts: [p, ko, cols] with p the SBUF partition
    def sb_pack(wT):  # [D, cols] -> [128, KO, cols]
        return wT.reshape(KO, 128, wT.shape[1]).transpose(1, 0, 2)

    xP_b = [
        _bf16(
            x[b].T.reshape(KO, 128, NCH, CH).transpose(2, 1, 0, 3)
        )
        for b in range(B)
    ]

    in_maps = []
    for c in range(N_CORES):
        b = c // (N_CORES // B)
        g = c % (N_CORES // B)
        es = g * E

        def pack_qk(w):
            rows = w[es : es + E]  # [E, D]
            blocks = [
                rows[h0 * DK : (h0 + 1) * DK][perm] for h0 in range(HPC)
            ]
            return sb_pack(np.concatenate(blocks, axis=0).T)

        def pack_bias(bvec):
            sl = bvec[es : es + E].reshape(HPC, DK)
            return np.ascontiguousarray(sl[:, perm])

        # bqk: [DK, 2*HPC] with q-bias heads in cols 0:HPC, k in HPC:
        bqk = np.concatenate(
            [pack_bias(wq_b).T, pack_bias(wk_b).T], axis=1
        )
        wqkv = np.stack(
            [
                pack_qk(wq_w),
                pack_qk(wk_w),
                sb_pack(wv_w[es : es + E].T),
            ],
            axis=0,
        )

        in_maps.append(
            {
                "xP": xP_b[b],
                "wqkv": _bf16(wqkv),
                "woT": _bf16(
                    wo_w[:, es : es + E].T.reshape(HPC, 128, D).transpose(
                        1, 0, 2
                    )
                ),
                "bqk": np.ascontiguousarray(bqk),
                "aux": aux_np,
            }
        )

    trace = bool(os.environ.get("MHA_TRACE"))
    res = run_bass_kernel_spmd(
        nc, in_maps, list(range(N_CORES)), trace=trace
    )
    last_exec_time_ns = res.exec_time_ns
    last_results = res

    # host-side gather: sum partials per batch, add biases that commute
    # with attention (softmax rows sum to 1, so wv_b passes straight
    # through to the output projection)
    const_bias = wo_b + wo_w @ wv_b  # [D]
    out = np.empty((B, S, D), dtype=np.float32)
    gpb = N_CORES // B
    for b in range(B):
        acc = res.results[b * gpb]["out"].astype(np.float32)
        for c in range(b * gpb + 1, (b + 1) * gpb):
            acc += res.results[c]["out"].astype(np.float32)
        out[b] = acc + const_bias[None, :]
    return out
